# revision 13
# baseline (speedup 1.0000x reference)
"""BiMambaBlock kernel for 8 TRN2 NeuronCores (Bass/Tile via PJRT).

Sharding: 8 cores = (modality i, direction dir, batch b) - each core runs the
full per-sequence pipeline on one channel-shuffled (and, for dir=1, L-flipped)
sequence x_i[b] of shape (96, 9216):
  LayerNorm -> RMSNorm -> fused causal-conv+input-projection -> SiLU
  -> B/C/dt projections -> selective scan (DVE tensor_tensor_scan over
  (d,n)-partition tiles, chunked along L with carried state) ->
  y = (scan + xc*Dp) * silu(z) -> 0.5 * output projection (+ residual on the
  fwd core). Host sums fwd/bwd partials and reshapes.

Perf notes vs the first version:
  - all heavy PE matmuls run fp32r (1 cycle/row at >=256 free) with
    producer-side F32R rounding; B/C one-hot broadcasts are single fp32r
    matmuls hoisted out of the hf loop (B/C are shared across the two
    d-halves).
  - B/C broadcast rows are copied PSUM->SBUF as bf16 so the big per-(n,hf)
    dtx*B and h*C multiplies run as all-bf16 TensorTensor ops in the DVE 2x
    mode; the scan emits bf16 and the y-accumulation over n uses a bf16
    identity matmul into PSUM.
  - a post-compile pass collapses the alternating exp/ln activation-table
    loads (2 per Ln<->Exp transition, ~145 total) into one load of the
    natural_log_exp_and_others set that contains every function used.
  - silu's +1 and the gated y multiply run on the otherwise idle GpSimd
    engine.

Self-contained: only needs numpy + jax + the concourse stack at
/opt/trn_rl_repo (present in the execution container).
"""
import sys
for _p in ("/opt/trn_rl_repo",):
    if _p not in sys.path:
        sys.path.insert(0, _p)
import numpy as np
from contextlib import ExitStack

import concourse.bass as bass
import concourse.bacc as bacc
import concourse.tile as tile
from concourse import mybir

F32 = mybir.dt.float32
F32R = mybir.dt.float32r
BF16 = mybir.dt.bfloat16
AF = mybir.ActivationFunctionType
OP = mybir.AluOpType

C, DI, N, R, K = 96, 192, 16, 6, 4
HH = WW = 96
L_FULL = HH * WW     # 9216
EPS = 1e-5

TC = 768             # time chunk
SUB = 384            # psum sub-chunk


def _fix_act_tables(nc):
    """Replace the alternating exp/ln table loads with a single load of a
    set containing every activation function the program uses."""
    from concourse.hw_specs import get_activation_tables
    used = set()
    for b in nc.main_func.blocks:
        for i in b.instructions:
            if isinstance(i, mybir.InstActivation):
                used.add(i.func)
    tables = get_activation_tables(nc.m.arch)
    target = None
    for idx, (name, funcs) in enumerate(tables.items()):
        if used <= funcs:
            target = idx
            break
    if target is None:
        return  # no single covering set; leave the compiler's placement
    for b in nc.main_func.blocks:
        keep_done = False
        new_insts = []
        for i in b.instructions:
            if isinstance(i, mybir.InstLoadActFuncSet):
                si = i.sync_info
                clean = si is None or (len(si.on_wait) == 0
                                       and len(si.on_update) == 0)
                if not keep_done or not clean:
                    i.act_func_set_id = target
                    keep_done = True
                    new_insts.append(i)
                # else: drop redundant load
            else:
                new_insts.append(i)
        b.instructions[:] = new_insts


def build_program(L=L_FULL, Tc=TC, pow_dA=True):
    NCHUNK = L // Tc
    NSUB = Tc // SUB
    TC3 = Tc + 3
    nc = bacc.Bacc("TRN2", target_bir_lowering=False, debug=False)

    x_in = nc.dram_tensor("x", [C, L], F32, kind="ExternalInput")
    wIN = nc.dram_tensor("wIN", [C, K * DI], F32, kind="ExternalInput")
    wZ = nc.dram_tensor("wZ", [C, DI], F32, kind="ExternalInput")
    wXP = nc.dram_tensor("wXP", [C, 2 * 38], F32, kind="ExternalInput")
    wDT = nc.dram_tensor("wDT", [R, DI], F32, kind="ExternalInput")
    wA = nc.dram_tensor("wA", [C, 2 * N], F32, kind="ExternalInput")
    wOUT = nc.dram_tensor("wOUT", [C, 2 * C], F32, kind="ExternalInput")
    vec2 = nc.dram_tensor("vec2", [C, 8], F32, kind="ExternalInput")
    vec1 = nc.dram_tensor("vec1", [C, 3], F32, kind="ExternalInput")
    gate_in = nc.dram_tensor("gate", [1, 1], F32, kind="ExternalInput")
    eye_in = nc.dram_tensor("eye", [C, C], BF16, kind="ExternalInput")
    sel_in = nc.dram_tensor("sel", [N, N * C], F32R, kind="ExternalInput")

    p_out = nc.dram_tensor("p", [C, L], F32, kind="ExternalOutput")

    with ExitStack() as ctx:
        tc = ctx.enter_context(tile.TileContext(nc))
        wp = ctx.enter_context(tc.tile_pool(name="wts", bufs=1))
        px = ctx.enter_context(tc.tile_pool(name="px", bufs=2))
        ph = ctx.enter_context(tc.tile_pool(name="ph", bufs=2))
        pt0 = ctx.enter_context(tc.tile_pool(name="pt0", bufs=2))
        pt1 = ctx.enter_context(tc.tile_pool(name="pt1", bufs=2))
        psp = ctx.enter_context(tc.tile_pool(name="psp", bufs=2))
        prow = ctx.enter_context(tc.tile_pool(name="prow", bufs=2))
        pxc = ctx.enter_context(tc.tile_pool(name="pxc", bufs=2))
        pg = ctx.enter_context(tc.tile_pool(name="pg", bufs=2))
        pdbl = ctx.enter_context(tc.tile_pool(name="pdbl", bufs=2))
        pdt = ctx.enter_context(tc.tile_pool(name="pdt", bufs=2))
        pdtx = ctx.enter_context(tc.tile_pool(name="pdtx", bufs=2))
        pbb = ctx.enter_context(tc.tile_pool(name="pbb", bufs=2))
        psc = ctx.enter_context(tc.tile_pool(name="psc", bufs=2))
        phh = ctx.enter_context(tc.tile_pool(name="phh", bufs=2))
        phc = ctx.enter_context(tc.tile_pool(name="phc", bufs=2))
        pst = ctx.enter_context(tc.tile_pool(name="pst", bufs=1))
        ptail = ctx.enter_context(tc.tile_pool(name="ptail", bufs=2))
        ppr = ctx.enter_context(tc.tile_pool(name="ppr", bufs=1))

        qh = ctx.enter_context(tc.tile_pool(name="qh", bufs=2, space="PSUM"))
        qbc = ctx.enter_context(tc.tile_pool(name="qbc", bufs=2, space="PSUM"))
        qy = ctx.enter_context(tc.tile_pool(name="qy", bufs=1, space="PSUM"))

        w_in = wp.tile([C, K * DI], F32); nc.sync.dma_start(w_in[:], wIN[:])
        w_z = wp.tile([C, DI], F32); nc.sync.dma_start(w_z[:], wZ[:])
        w_xp = wp.tile([C, 2 * 38], F32); nc.sync.dma_start(w_xp[:], wXP[:])
        w_dt = wp.tile([R, DI], F32); nc.sync.dma_start(w_dt[:], wDT[:])
        w_a = wp.tile([C, 2 * N], F32); nc.sync.dma_start(w_a[:], wA[:])
        w_out = wp.tile([C, 2 * C], F32); nc.sync.dma_start(w_out[:], wOUT[:])
        v2 = wp.tile([C, 8], F32); nc.sync.dma_start(v2[:], vec2[:])
        v1 = wp.tile([C, 3], F32); nc.sync.dma_start(v1[:], vec1[:])
        gt = wp.tile([1, 1], F32); nc.sync.dma_start(gt[:], gate_in[:])
        eye_b = wp.tile([C, C], BF16); nc.sync.dma_start(eye_b[:], eye_in[:])

        # fp32r-rounded copies of the stationary matmul operands
        w_inr = wp.tile([C, K * DI], F32R); nc.scalar.copy(w_inr[:], w_in[:])
        w_zr = wp.tile([C, DI], F32R); nc.scalar.copy(w_zr[:], w_z[:])
        w_xpr = wp.tile([C, 2 * 38], F32R); nc.scalar.copy(w_xpr[:], w_xp[:])
        w_dtr = wp.tile([R, DI], F32R); nc.scalar.copy(w_dtr[:], w_dt[:])
        w_outr = wp.tile([C, 2 * C], F32R); nc.scalar.copy(w_outr[:], w_out[:])
        sel_r = wp.tile([N, N * C], F32R); nc.sync.dma_start(sel_r[:], sel_in[:])

        ones_col = wp.tile([C, 1], F32); nc.vector.memset(ones_col[:], 1.0)
        ones_col_r = wp.tile([C, 1], F32R); nc.scalar.copy(ones_col_r[:],
                                                          ones_col[:])
        ones_row = wp.tile([1, C], F32); nc.vector.memset(ones_row[:], 1.0)
        ones_row_r = wp.tile([1, C], F32R); nc.scalar.copy(ones_row_r[:],
                                                          ones_row[:])
        epsc = wp.tile([1, 1], F32); nc.vector.memset(epsc[:], EPS)
        gate_c = wp.tile([C, 1], F32)
        qg = qh.tile([C, 1], F32, tag="a")
        nc.tensor.matmul(qg[:], ones_row[:], gt[:])
        nc.scalar.copy(gate_c[:], qg[:])

        # probes: absorb cross-engine waits (TensorScalarPtr ops: 1 wait slot)
        prv = ppr.tile([1, 8], F32)
        pra = ppr.tile([1, 8], F32)
        nc.vector.tensor_copy(prv[:, 0:1], v1[:1, 0:1])
        nc.vector.tensor_copy(prv[:, 1:2], v2[:1, 0:1])
        nc.vector.tensor_copy(prv[:, 2:3], gate_c[:1, 0:1])
        nc.scalar.copy(pra[:, 0:1], w_a[:1, 0:1])
        nc.scalar.copy(pra[:, 1:2], v1[:1, 0:1])
        nc.scalar.copy(pra[:, 2:3], v2[:1, 0:1])

        st = pst.tile([C, 2 * N], F32)
        nc.vector.memset(st[:], 0.0)
        zero3 = wp.tile([C, 3], F32); nc.vector.memset(zero3[:], 0.0)

        h_prev = None
        def make_prologue(ci, chunks):
            """Emit-later closures for chunk ci's pre-scan pipeline. Each
            stage is emitted interleaved with the previous chunk's scan
            iterations so the in-order engine queues overlap them."""
            S = {"t0": ci * Tc}
            t0 = S["t0"]

            def s_dma():
                S["xt"] = px.tile([C, Tc], F32, tag="xt", name="xt")
                nc.sync.dma_start(S["xt"][:], x_in[:, t0:t0 + Tc])

            def s_sq():
                S["sq"] = pt0.tile([C, Tc], F32R, tag="sq", name="sq")
                nc.scalar.activation(S["sq"][:], S["xt"][:], AF.Square)

            def s_stats():
                S["m_"] = prow.tile([1, Tc], F32R, tag="m", bufs=2, name="m_")
                S["var_"] = prow.tile([1, Tc], F32, tag="var", bufs=1,
                                      name="var_")
                mm_ = prow.tile([1, Tc], F32, tag="mm", bufs=1, name="mm_")
                for si in range(NSUB):
                    o = si * SUB
                    s1 = qh.tile([1, SUB], F32, tag="a", name="s1")
                    nc.tensor.matmul(s1[:], ones_col[:], S["xt"][:, o:o + SUB])
                    nc.vector.tensor_scalar_mul(
                        S["m_"][:, o:o + SUB], s1[:], 1.0 / C)
                    s2 = qh.tile([1, SUB], F32, tag="a", name="s2")
                    nc.tensor.matmul(s2[:], ones_col_r[:],
                                     S["sq"][:, o:o + SUB])
                    nc.vector.tensor_tensor(
                        mm_[:, o:o + SUB], S["m_"][:, o:o + SUB].bitcast(F32),
                        S["m_"][:, o:o + SUB].bitcast(F32), op=OP.mult)
                    nc.vector.scalar_tensor_tensor(
                        S["var_"][:, o:o + SUB], s2[:], 1.0 / C,
                        mm_[:, o:o + SUB], op0=OP.mult, op1=OP.subtract)

            def s_rstd():
                lnv = prow.tile([1, Tc], F32, tag="lnv", bufs=1, name="lnv")
                S["rstd"] = prow.tile([1, Tc], F32R, tag="rstd", bufs=2,
                                      name="rstd")
                nc.scalar.activation(lnv[:], S["var_"][:], AF.Ln,
                                     bias=epsc[:, 0:1])
                nc.scalar.activation(S["rstd"][:], lnv[:], AF.Exp, scale=-0.5)

            def s_xn():
                S["xn"] = pt1.tile([C, Tc], F32, tag="xn", name="xn")
                for si in range(NSUB):
                    o = si * SUB
                    mb = qh.tile([C, SUB], F32, tag="a", name="mb")
                    nc.tensor.matmul(mb[:], ones_row_r[0:1, :],
                                     S["m_"][:, o:o + SUB])
                    nc.vector.tensor_tensor(S["xn"][:, o:o + SUB],
                                            S["xt"][:, o:o + SUB],
                                            mb[:], op=OP.subtract)
                    rb = qh.tile([C, SUB], F32, tag="a", name="rb")
                    nc.tensor.matmul(rb[:], ones_row_r[0:1, :],
                                     S["rstd"][:, o:o + SUB])
                    nc.vector.tensor_tensor(S["xn"][:, o:o + SUB],
                                            S["xn"][:, o:o + SUB],
                                            rb[:], op=OP.mult)

            def s_ln():
                S["ln_t"] = pt1.tile([C, Tc], F32, tag="ln", name="ln_t")
                nc.scalar.activation(S["ln_t"][:], S["xn"][:], AF.Identity,
                                     bias=v1[:, 1:2], scale=v1[:, 0:1])
                S["lsq"] = pt0.tile([C, Tc], F32R, tag="lsq", name="lsq")
                nc.scalar.activation(S["lsq"][:], S["ln_t"][:], AF.Square)

            def s_rr():
                lnr = prow.tile([1, Tc], F32, tag="lnr", bufs=1, name="lnr")
                S["rr"] = prow.tile([1, Tc], F32R, tag="rr", bufs=2, name="rr")
                for si in range(NSUB):
                    o = si * SUB
                    s3 = qh.tile([1, SUB], F32, tag="a", name="s3")
                    nc.tensor.matmul(s3[:], ones_col_r[:],
                                     S["lsq"][:, o:o + SUB])
                    nc.scalar.activation(lnr[:, o:o + SUB], s3[:],
                                         AF.Ln, scale=1.0 / C,
                                         bias=epsc[:, 0:1])
                    nc.scalar.activation(S["rr"][:, o:o + SUB],
                                         lnr[:, o:o + SUB], AF.Exp, scale=-0.5)

            def s_h():
                h_t = ph.tile([C, TC3], F32R, tag="h", name="h_t")
                S["h_t"] = h_t
                if ci == 0:
                    nc.scalar.copy(h_t[:, 0:3], zero3[:])
                else:
                    hp = chunks[ci - 1]["h_t"]
                    nc.vector.tensor_copy(h_t[:, 0:3],
                                          hp[:, Tc:Tc + 3].bitcast(F32))
                nc.vector.tensor_copy(prv[:, 3:4], S["ln_t"][:1, 0:1])
                for si in range(NSUB):
                    o = si * SUB
                    rrb = qh.tile([C, SUB], F32, tag="a", name="rrb")
                    nc.tensor.matmul(rrb[:], ones_row_r[0:1, :],
                                     S["rr"][:, o:o + SUB])
                    nc.vector.scalar_tensor_tensor(
                        h_t[:, 3 + o:3 + o + SUB], S["ln_t"][:, o:o + SUB],
                        v1[:, 2:3], rrb[:], op0=OP.mult, op1=OP.mult)

            def make_conv(hf):
                def s_conv():
                    if "xc_h" not in S:
                        S["xc_h"] = [None, None]
                        S["g_h"] = [None, None]
                    xc = pxc.tile([C, Tc], F32R, tag=f"xc{hf}", name=f"xc{hf}")
                    g = pg.tile([C, Tc], F32, tag=f"g{hf}", name=f"g{hf}")
                    S["xc_h"][hf] = xc
                    S["g_h"][hf] = g
                    h_t = S["h_t"]
                    for si in range(NSUB):
                        o = si * SUB
                        ps = qh.tile([C, SUB], F32, tag="a", name="psc1")
                        for k in range(K):
                            nc.tensor.matmul(
                                ps[:],
                                w_inr[:, k * DI + hf * C:k * DI + hf * C + C],
                                h_t[:, o + k:o + k + SUB],
                                start=(k == 0), stop=(k == K - 1))
                        # silu(p+cb) = (p+cb) / (1+exp(-(p+cb)))
                        e1 = psp.tile([C, SUB], F32, tag="sg1", name="e1")
                        nc.scalar.activation(e1[:], ps[:], AF.Exp, scale=-1.0,
                                             bias=v2[:, 6 + hf:7 + hf])
                        nc.gpsimd.tensor_scalar_add(e1[:], e1[:], 1.0)
                        r1 = psp.tile([C, SUB], F32, tag="sg2", name="r1")
                        nc.vector.reciprocal_approx_fast(r1[:], e1[:])
                        nc.vector.scalar_tensor_tensor(
                            xc[:, o:o + SUB], ps[:], v2[:, hf:hf + 1], r1[:],
                            op0=OP.add, op1=OP.mult)
                        ps2 = qh.tile([C, SUB], F32, tag="a", name="psc2")
                        nc.tensor.matmul(ps2[:], w_zr[:, hf * C:hf * C + C],
                                         h_t[:, o + 3:o + 3 + SUB])
                        e2 = psp.tile([C, SUB], F32, tag="sg1", name="e2")
                        nc.scalar.activation(e2[:], ps2[:], AF.Exp, scale=-1.0)
                        nc.gpsimd.tensor_scalar_add(e2[:], e2[:], 1.0)
                        r2 = psp.tile([C, SUB], F32, tag="sg2", name="r2")
                        nc.vector.reciprocal_approx_fast(r2[:], e2[:])
                        nc.vector.tensor_tensor(g[:, o:o + SUB], ps2[:], r2[:],
                                                op=OP.mult)
                return s_conv

            def s_dbl():
                # B/C/dt projections; contraction over the full DI
                S["dproj"] = pdbl.tile([R, Tc], F32R, tag="dproj",
                                       name="dproj")
                S["b_t"] = pdbl.tile([N, Tc], F32R, tag="b_t", name="b_t")
                S["c_t"] = pdbl.tile([N, Tc], F32R, tag="c_t", name="c_t")
                for si in range(NSUB):
                    o = si * SUB
                    for lo, sz, dst in ((R, N, S["b_t"]), (R + N, N, S["c_t"]),
                                        (0, R, S["dproj"])):
                        ps = qh.tile([sz, SUB], F32, tag="a", name="psdbl")
                        for hf in range(2):
                            nc.tensor.matmul(
                                ps[:],
                                w_xpr[:, hf * 38 + lo:hf * 38 + lo + sz],
                                S["xc_h"][hf][:, o:o + SUB],
                                start=(hf == 0), stop=(hf == 1))
                        nc.scalar.copy(dst[:, o:o + SUB], ps[:])

            def s_dt():
                S["dt_h"] = [pdt.tile([C, Tc], F32, tag=f"dt{hf}",
                                      name=f"dt{hf}") for hf in range(2)]
                for hf in range(2):
                    for si in range(NSUB):
                        o = si * SUB
                        ps = qh.tile([C, SUB], F32, tag="a", name="psdt")
                        nc.tensor.matmul(ps[:], w_dtr[:, hf * C:hf * C + C],
                                         S["dproj"][0:R, o:o + SUB])
                        # softplus: dt projections sit near dtb ~ -4, so the
                        # direct ln(1+exp(v)) form cannot overflow
                        ex = psp.tile([C, SUB], F32, tag="spe", name="ex")
                        nc.scalar.activation(ex[:], ps[:], AF.Exp,
                                             bias=v2[:, 2 + hf:3 + hf])
                        nc.scalar.activation(S["dt_h"][hf][:, o:o + SUB],
                                             ex[:], AF.Ln, bias=1.0)

            def s_dtx():
                S["dtx_h"] = []
                S["r_h"] = []
                for hf in range(2):
                    dx = pdtx.tile([C, Tc], BF16, tag=f"dtx{hf}",
                                   name=f"dtx{hf}")
                    nc.gpsimd.tensor_tensor(dx[:], S["dt_h"][hf][:],
                                            S["xc_h"][hf][:].bitcast(F32),
                                            op=OP.mult)
                    S["dtx_h"].append(dx)
                    if pow_dA:
                        rt = psc.tile([C, Tc], F32, tag=f"r{hf}",
                                      name=f"r{hf}")
                        nc.scalar.activation(rt[:], S["dt_h"][hf][:], AF.Exp,
                                             scale=w_a[:, hf:hf + 1])
                        S["r_h"].append(rt)

            return S, [s_dma, s_sq, s_stats, s_rstd, s_xn, s_ln, s_rr, s_h,
                       make_conv(0), make_conv(1), s_dbl, s_dt, s_dtx]

        def emit_scan_and_tail(S, nxt_stages):
            t0 = S["t0"]
            xt, xc_h, g_h = S["xt"], S["xc_h"], S["g_h"]
            dt_h, dtx_h, r_h = S["dt_h"], S["dtx_h"], S["r_h"]
            b_t, c_t = S["b_t"], S["c_t"]
            yps = [[qy.tile([C, SUB], F32, tag=f"y{hf}_{si}",
                            name=f"y{hf}_{si}")
                    for si in range(NSUB)] for hf in range(2)]
            dA_prev = [None, None]
            for n in range(N):
                bb = pbb.tile([C, Tc], BF16, tag="bb", name="bb")
                cb = pbb.tile([C, Tc], BF16, tag="cb", name="cb")
                for si in range(NSUB):
                    o = si * SUB
                    bps = qbc.tile([C, SUB], F32, tag="bc", name="bps")
                    nc.tensor.matmul(bps[:], sel_r[:, n * C:(n + 1) * C],
                                     b_t[:, o:o + SUB])
                    nc.scalar.copy(bb[:, o:o + SUB], bps[:])
                    cps = qbc.tile([C, SUB], F32, tag="bc", name="cps")
                    nc.tensor.matmul(cps[:], sel_r[:, n * C:(n + 1) * C],
                                     c_t[:, o:o + SUB])
                    nc.scalar.copy(cb[:, o:o + SUB], cps[:])
                for hf in range(2):
                    idx = n * 2 + hf
                    if pow_dA:
                        if n == 0:
                            dA = r_h[hf]
                        else:
                            dA = psc.tile([C, Tc], F32, tag=f"dA{hf}",
                                          name=f"dA{hf}")
                            nc.gpsimd.tensor_tensor(dA[:], dA_prev[hf][:],
                                                    r_h[hf][:], op=OP.mult)
                        dA_prev[hf] = dA
                    else:
                        dA = psc.tile([C, Tc], F32, tag="dA", name="dA")
                        nc.scalar.activation(dA[:], dt_h[hf][:], AF.Exp,
                                             scale=w_a[:, idx:idx + 1])
                    bt = psc.tile([C, Tc], BF16, tag="bt", name="bt")
                    nc.gpsimd.tensor_tensor(bt[:], dtx_h[hf][:], bb[:],
                                            op=OP.mult)
                    ht = phh.tile([C, Tc], BF16, tag="ht", name="ht")
                    nc.vector.tensor_tensor_scan(
                        ht[:], dA[:], bt[:], st[:, idx:idx + 1],
                        op0=OP.mult, op1=OP.add)
                    nc.vector.tensor_copy(st[:, idx:idx + 1], ht[:, Tc - 1:Tc])
                    hc = phc.tile([C, Tc], BF16, tag="hc", name="hc")
                    if hf == 0:
                        nc.gpsimd.tensor_tensor(hc[:], ht[:], cb[:],
                                                op=OP.mult)
                    else:
                        nc.vector.tensor_tensor(hc[:], ht[:], cb[:],
                                                op=OP.mult)
                    for si in range(NSUB):
                        o = si * SUB
                        nc.tensor.matmul(yps[hf][si][:], eye_b[:],
                                         hc[:, o:o + SUB],
                                         start=(n == 0), stop=(n == N - 1),
                                         skip_group_check=True)
                if nxt_stages:
                    nxt_stages.pop(0)()

            while nxt_stages:
                nxt_stages.pop(0)()
            yg_h = []
            for hf in range(2):
                ya = ptail.tile([C, Tc], F32, tag=f"ya{hf}", name=f"ya{hf}",
                                bufs=1)
                for si in range(NSUB):
                    o = si * SUB
                    nc.vector.scalar_tensor_tensor(
                        ya[:, o:o + SUB], xc_h[hf][:, o:o + SUB].bitcast(F32),
                        v2[:, 4 + hf:5 + hf], yps[hf][si][:],
                        op0=OP.mult, op1=OP.add)
                yg = ptail.tile([C, Tc], F32R, tag=f"yg{hf}", name=f"yg{hf}",
                                bufs=1)
                nc.gpsimd.tensor_tensor(yg[:], ya[:], g_h[hf][:], op=OP.mult)
                yg_h.append(yg)
            for si in range(NSUB):
                o = si * SUB
                pso = qh.tile([C, SUB], F32, tag="a", name="pso")
                for hf in range(2):
                    nc.tensor.matmul(pso[:], w_outr[:, hf * C:hf * C + C],
                                     yg_h[hf][:, o:o + SUB],
                                     start=(hf == 0), stop=(hf == 1))
                ot = ptail.tile([C, SUB], F32, tag="ot", name="ot")
                nc.vector.scalar_tensor_tensor(
                    ot[:], xt[:, o:o + SUB], gate_c[:, 0:1], pso[:],
                    op0=OP.mult, op1=OP.add)
                nc.sync.dma_start(p_out[:, t0 + o:t0 + o + SUB], ot[:])

        chunks = {}
        S0, st0 = make_prologue(0, chunks)
        chunks[0] = S0
        for f in st0:
            f()
        for ci in range(NCHUNK):
            if ci + 1 < NCHUNK:
                S1, stages = make_prologue(ci + 1, chunks)
                chunks[ci + 1] = S1
            else:
                stages = []
            emit_scan_and_tail(chunks[ci], stages)
            chunks.pop(ci - 1, None)

    nc.compile()
    _fix_act_tables(nc)
    return nc


# ---------------------------------------------------------------- host side

def shuffle_channels(x):
    c = x.shape[0]
    return x.reshape(2, c // 2, -1).transpose(1, 0, 2).reshape(c, -1)


def pack_core_inputs(i, dr, b, x1, x2, inw, convw, convb, xpw, dtw, dtb,
                     Alog, Dp, outw, rmsw, lnw, lnb):
    xs = x1 if i == 0 else x2
    x = shuffle_channels(np.asarray(xs[b], np.float32))
    if dr == 1:
        x = x[:, ::-1]
    x = np.ascontiguousarray(x)

    inw_i = np.asarray(inw[i], np.float32)
    cw = np.asarray(convw[i, dr], np.float32)
    cb = np.asarray(convb[i, dr], np.float32)
    xp = np.asarray(xpw[i, dr], np.float32)
    dw = np.asarray(dtw[i, dr], np.float32)
    db = np.asarray(dtb[i, dr], np.float32)
    Av = -np.exp(np.asarray(Alog[i, dr], np.float32))
    Dv = np.asarray(Dp[i, dr], np.float32)
    ow = np.asarray(outw[i], np.float32)

    wIN = np.empty((C, K * DI), np.float32)
    inw_x = inw_i[:DI]
    for k in range(K):
        wIN[:, k * DI:(k + 1) * DI] = (cw[:, k][:, None] * inw_x).T
    wZ = np.ascontiguousarray(inw_i[DI:].T)
    wXP = np.empty((C, 2 * 38), np.float32)
    for hf in range(2):
        wXP[:, hf * 38:(hf + 1) * 38] = xp[:, hf * C:(hf + 1) * C].T
    wDT = np.ascontiguousarray(dw.T)
    wA = np.empty((C, 2 * N), np.float32)
    for nn in range(N):
        for hf in range(2):
            wA[:, nn * 2 + hf] = Av[hf * C:(hf + 1) * C, nn]
    wOUT = np.empty((C, 2 * C), np.float32)
    for hf in range(2):
        wOUT[:, hf * C:(hf + 1) * C] = 0.5 * ow[:, hf * C:(hf + 1) * C].T
    vec2 = np.ascontiguousarray(
        np.stack([cb[:C], cb[C:], db[:C], db[C:], Dv[:C], Dv[C:],
                  -cb[:C], -cb[C:]], axis=1), dtype=np.float32)
    vec1 = np.ascontiguousarray(
        np.stack([np.asarray(lnw[i], np.float32),
                  np.asarray(lnb[i], np.float32),
                  np.asarray(rmsw[i], np.float32)], axis=1), dtype=np.float32)
    gate = np.array([[1.0 if dr == 0 else 0.0]], np.float32)
    try:
        import ml_dtypes
        _bf16 = ml_dtypes.bfloat16
    except Exception:
        import jax.numpy as _jnp
        _bf16 = _jnp.bfloat16
    eye = np.eye(C, dtype=_bf16)
    sel = np.zeros((N, N * C), np.float32)
    for nn in range(N):
        sel[nn, nn * C:(nn + 1) * C] = 1.0
    return {
        "x": x, "wIN": wIN, "wZ": wZ, "wXP": wXP, "wDT": wDT, "wA": wA,
        "wOUT": wOUT, "vec2": vec2, "vec1": vec1, "gate": gate, "eye": eye,
        "sel": sel,
    }


def make_in_maps(inputs):
    args = dict(
        x1=np.asarray(inputs["x1"], np.float32),
        x2=np.asarray(inputs["x2"], np.float32),
        inw=np.asarray(inputs["inw"], np.float32),
        convw=np.asarray(inputs["convw"], np.float32),
        convb=np.asarray(inputs["convb"], np.float32),
        xpw=np.asarray(inputs["xpw"], np.float32),
        dtw=np.asarray(inputs["dtw"], np.float32),
        dtb=np.asarray(inputs["dtb"], np.float32),
        Alog=np.asarray(inputs["Alog"], np.float32),
        Dp=np.asarray(inputs["Dp"], np.float32),
        outw=np.asarray(inputs["outw"], np.float32),
        rmsw=np.asarray(inputs["rmsw"], np.float32),
        lnw=np.asarray(inputs["lnw"], np.float32),
        lnb=np.asarray(inputs["lnb"], np.float32),
    )
    in_maps, core_meta = [], []
    for i in range(2):
        for dr in range(2):
            for b in range(2):
                in_maps.append(pack_core_inputs(i, dr, b, **args))
                core_meta.append((i, dr, b))
    return in_maps, core_meta


def assemble_outputs(results, core_meta):
    B = 2
    outs = []
    for i in range(2):
        acc = np.zeros((B, C, L_FULL), np.float32)
        for (ii, dr, b), res in zip(core_meta, results):
            if ii != i:
                continue
            p = res["p"]
            if dr == 1:
                p = p[:, ::-1]
            acc[b] += p
        outs.append(acc.reshape(B, C, HH, WW))
    return tuple(outs)


# ------------------------------------------------------------- PJRT executor

class _BassExec:
    def __init__(self, nc, n_cores):
        import jax
        from jax.sharding import Mesh, PartitionSpec
        from jax.experimental.shard_map import shard_map
        from concourse.bass2jax import (_bass_exec_p, install_neuronx_cc_hook,
                                        partition_id_tensor)
        install_neuronx_cc_hook()
        self.jax = jax
        self.n_cores = n_cores
        partition_name = (nc.partition_id_tensor.name
                          if nc.partition_id_tensor else None)
        in_names, out_names, out_avals, zero_outs = [], [], [], []
        for alloc in nc.m.functions[0].allocations:
            if not isinstance(alloc, mybir.MemoryLocationSet):
                continue
            name = alloc.memorylocations[0].name
            if alloc.kind == "ExternalInput":
                if name != partition_name:
                    in_names.append(name)
            elif alloc.kind == "ExternalOutput":
                shape = tuple(alloc.tensor_shape)
                dtype = mybir.dt.np(alloc.dtype)
                out_names.append(name)
                out_avals.append(jax.core.ShapedArray(shape, dtype))
                zero_outs.append(np.zeros(shape, dtype))
        self.in_names, self.out_names = in_names, out_names
        self.out_avals, self.zero_outs = out_avals, zero_outs
        n_params, n_outs = len(in_names), len(out_avals)
        bind_names = in_names + out_names + ([partition_name] if partition_name
                                             else [])

        def _body(*args):
            operands = list(args)
            if partition_name is not None:
                operands.append(partition_id_tensor())
            outs = _bass_exec_p.bind(
                *operands,
                out_avals=tuple(out_avals),
                in_names=tuple(bind_names),
                out_names=tuple(out_names),
                lowering_input_output_aliases=(),
                sim_require_finite=True,
                sim_require_nnan=True,
                nc=nc,
            )
            return tuple(outs)

        devices = jax.devices()[:n_cores]
        self.mesh = Mesh(np.asarray(devices), ("core",))
        in_specs = (PartitionSpec("core"),) * (n_params + n_outs)
        out_specs = (PartitionSpec("core"),) * n_outs
        self.fn = jax.jit(
            shard_map(_body, mesh=self.mesh, in_specs=in_specs,
                      out_specs=out_specs, check_rep=False),
            keep_unused=True)

    def prep(self, in_maps):
        from jax.sharding import NamedSharding, PartitionSpec
        concat_in = [
            np.concatenate([np.asarray(in_maps[c][n])
                            for c in range(self.n_cores)], axis=0)
            for n in self.in_names
        ]
        concat_zero = [
            np.zeros((self.n_cores * z.shape[0], *z.shape[1:]), z.dtype)
            for z in self.zero_outs
        ]
        sh = NamedSharding(self.mesh, PartitionSpec("core"))
        return [self.jax.device_put(a, sh) for a in concat_in + concat_zero]

    def run(self, args):
        outs = self.fn(*args)
        self.jax.block_until_ready(outs)
        return outs

    def results(self, outs):
        res = []
        for c in range(self.n_cores):
            m = {}
            for i, name in enumerate(self.out_names):
                a = np.asarray(outs[i])
                a = a.reshape(self.n_cores, *self.out_avals[i].shape)[c]
                m[name] = a
            res.append(m)
        return res


_CACHE = {}


def _get_exec(pow_dA=True):
    key = f"ex{int(pow_dA)}"
    if key not in _CACHE:
        nc = build_program(pow_dA=pow_dA)
        _CACHE[key] = _BassExec(nc, 8)
    return _CACHE[key]


def _alog_is_arange(Alog):
    # dA power chain is valid iff A[:, n] == A[:, 0] * (n+1) for every
    # (modality, direction); setup_inputs uses Alog = log(arange(1, N+1))
    # broadcast, which satisfies this exactly.
    Av = -np.exp(np.asarray(Alog, np.float64).reshape(-1, N))
    want = Av[:, 0:1] * np.arange(1, N + 1)[None, :]
    return np.allclose(Av, want, rtol=1e-5, atol=1e-7)


def kernel(**inputs):
    H = int(inputs.get("H", HH))
    W = int(inputs.get("W", WW))
    assert H == HH and W == WW, (H, W)
    in_maps, core_meta = make_in_maps(inputs)
    ex = _get_exec(pow_dA=_alog_is_arange(inputs["Alog"]))
    args = ex.prep(in_maps)
    outs = ex.run(args)
    res = ex.results(outs)
    return assemble_outputs(res, core_meta)
